# revision 1
# baseline (speedup 1.0000x reference)
"""Causal self-attention (B=4, T=2048, D=1024, H=16) on 8 Trainium2 cores.

Sharding: tensor-parallel over heads - 2 heads per core. Each core computes
its QKV shard, causal attention for its heads, and a partial output
projection; the host sums the 8 partials.

Key choices (vs the fp32 baseline this evolved from):
  - x and w_qkv are bf16 (host-converted) and the partial output is written
    as bf16: ~halves DMA traffic.  QKV matmuls run in bf16; the attention
    matmuls (scores / AV / out-proj) stay fp32r, which self-loads weights
    (bf16 matmuls cost an extra Ldweights sequencer instruction each).
  - 1024-wide q-chunks: one [128,1024] PSUM score tile per k-tile (two
    512-wide matmuls, ISA caps the moving dim at 512) and ONE exp per tile,
    halving the activation-engine instruction count.
  - causal mask is a single [128,128] lower-tri 0/1 tile applied
    multiplicatively in-place to the exp'd diagonal block (DVE), instead of
    an additive -1e9 mask + per-column mask tensor.
  - x chunk DMAs are prefetched one batch ahead so the SP DMA queue is not
    head-of-line blocked behind the previous batch's output DMAs.
  - ones columns of the V tile (denominator row-sum trick) come from one
    strided DMA; the V tile is double-buffered by batch parity so phase 1
    of batch b+1 can overlap attention of batch b.

Per-core dataflow (PSUM accum fp32 everywhere):
  phase 1 per 1024-tok chunk: xts DMA; q,k matmuls (contract d on
      partitions) -> qT/kT [feat, tok] SBUF; v feature-layout matmuls +
      PE transposes -> vv [k, kt, h, 0:64] with ones in cols 64:128.
  phase 2 per (1024-chunk, head): per k-tile one scores^T [k, live-q]
      PSUM tile; exp with fused 1/8 scale (ACT, PSUM->SBUF); diagonal
      128x128 block masked in-place; AV accumulation per 512-half with
      ones rows giving denominators on partitions 64:127; reciprocal *
      mult -> attnT.
  phase 3 per 512-tok block: out projection attnT^T x wout -> [tok, 1024]
      PSUM, copy to bf16 ob, one DMA per block. Host sums partials.
"""

import os
import sys

sys.path.insert(0, "/opt/trn_rl_repo")

import numpy as np
import ml_dtypes
from contextlib import ExitStack

import concourse.bass as bass
import concourse.mybir as mybir
import concourse.tile as tile
from concourse import bacc
from concourse.bass_utils import run_bass_kernel_spmd

B, T, D, H, HD = 4, 2048, 1024, 16, 64
NCORES = 8
HPC = H // NCORES          # heads per core = 2
DC = HPC * HD              # per-core feature width = 128
TOK = B * T                # 8192
TB = T // 128              # k-tiles per batch = 16
CW = 1024                  # q-chunk width
NCH = T // CW              # chunks per batch = 2
F32 = mybir.dt.float32
F32R = mybir.dt.float32r
BF16 = mybir.dt.bfloat16
EXP = mybir.ActivationFunctionType.Exp
SCALE = 1.0 / 8.0          # 1/sqrt(HD)

LAST_RESULTS = None


def _env(name, dflt):
    return os.environ.get(name, dflt)


QK_EV = _env("K_QK_EV", "s")       # qT/kT PSUM->SBUF copies: s=ACT, v=DVE
OB_EV = _env("K_OB_EV", "v")       # outproj copies: v=DVE, s=ACT, b=alternate
V_EV = _env("K_V_EV", "v")         # v transpose-pack copies: v=DVE, s=ACT
MASK_EV = _env("K_MASK_EV", "v")   # tri-mask mult: v=DVE, p=Pool
OUTDMA_EV = _env("K_OUTDMA_EV", "s")  # out DMA issue queue: p=Pool, s=SP
AV_DT = _env("K_AV_DT", "r")       # probs/V dtype: r=fp32r, b=bf16


def _copy(nc, ev, out, in_):
    if ev == "v":
        nc.vector.tensor_copy(out, in_)
    elif ev == "p":
        nc.gpsimd.tensor_copy(out, in_)
    else:
        nc.scalar.copy(out, in_)


def _attention_kernel(tc, out4, xTr, wqkvT, woutT, trimaskd, identd, vonesd):
    nc = tc.nc
    with ExitStack() as ctx:
        const = ctx.enter_context(tc.tile_pool(name="const", bufs=1))
        sbqk = ctx.enter_context(tc.tile_pool(name="sbqk", bufs=2))
        sbvv = ctx.enter_context(tc.tile_pool(name="sbvv", bufs=1))
        sbvt = ctx.enter_context(tc.tile_pool(name="sbvt", bufs=2))
        sbx = ctx.enter_context(tc.tile_pool(name="sbx", bufs=2))
        sbpt = ctx.enter_context(tc.tile_pool(name="sbpt", bufs=12))
        sba = ctx.enter_context(tc.tile_pool(name="sba", bufs=2))
        sbrc = ctx.enter_context(tc.tile_pool(name="sbrc", bufs=2))
        sbob = ctx.enter_context(tc.tile_pool(name="sbob", bufs=2))
        psS = ctx.enter_context(tc.tile_pool(name="psS", bufs=2, space="PSUM"))
        psAV = ctx.enter_context(tc.tile_pool(name="psAV", bufs=2, space="PSUM"))
        psOP = ctx.enter_context(tc.tile_pool(name="psOP", bufs=2, space="PSUM"))

        # ---- constants ----
        w_sb = const.tile([128, 8, 3 * DC], BF16, tag="wqkv")
        nc.sync.dma_start(out=w_sb, in_=wqkvT.rearrange("(dt p) f -> p dt f", p=128))
        wo_sb = const.tile([128, D], F32R, tag="wout")
        nc.sync.dma_start(out=wo_sb, in_=woutT)
        trimask = const.tile([128, 128], F32 if AV_DT == "r" else BF16,
                             tag="trimask")
        nc.sync.dma_start(out=trimask, in_=trimaskd)
        ident = const.tile([128, 128], F32R if AV_DT == "r" else BF16,
                           tag="ident")
        nc.sync.dma_start(out=ident, in_=identd)

        pools = (sbqk, sbvv, sbvt, sbx, sbpt, sba, sbrc, sbob, psS, psAV, psOP)

        # vv double-buffered manually (batch parity); ones cols written once
        # (memset on the idle Pool engine - keeps startup off the DMA path)
        # vv [128, kt, h, 128]: cols 0:64 V data (rewritten per batch), cols
        # 64:128 ones for the denominator row-sum trick (written once; Pool
        # memset via a f32-viewed AP keeps startup off the DMA path)
        vvs = []
        for pb in range(2):
            vv = sbvv.tile([128, TB, HPC, 128],
                           F32R if AV_DT == "r" else BF16, tag=f"vv{pb}")
            ones_ap = vv[:, :, :, 64:128]
            if AV_DT == "r":
                ones_ap = ones_ap.bitcast(F32)
            nc.gpsimd.memset(ones_ap, 1.0)
            vvs.append(vv)

        def body():
            _kernel_body(tc, out4, xTr, w_sb, wo_sb, trimask, ident, vvs, pools)

        nloop = int(os.environ.get("K_LOOP", "1"))
        if nloop > 1:
            with tc.For_i(0, nloop, 1):
                body()
        else:
            body()


def _kernel_body(tc, out4, xTr, w_sb, wo_sb, trimask, ident, vvs, pools):
    (sbqk, sbvv, sbvt, sbx, sbpt, sba, sbrc, sbob, psS, psAV, psOP) = pools
    nc = tc.nc

    # x chunk DMAs are issued one batch ahead so the SP queue isn't blocked
    # behind the previous batch's output DMAs when the next batch starts
    xts_tiles = {}

    def fetch_x(b):
        if b >= B:
            return
        for ci in range(NCH):
            xts = sbx.tile([128, 8, CW], BF16, tag="xts")
            nc.sync.dma_start(out=xts, in_=xTr[b * NCH + ci])
            xts_tiles[(b, ci)] = xts

    fetch_x(0)
    for b in range(B):
        vv = vvs[b % 2]
        # ================= phase 1: QKV projection =================
        qT = sbqk.tile([128, T], F32R, tag="qT")
        kT = sbqk.tile([128, T], F32R, tag="kT")
        for ci in range(NCH):                    # 1024-token chunks
            xts = xts_tiles.pop((b, ci))
            for ft, dst in ((0, qT), (1, kT)):
                qkp = psS.tile([128, CW], F32, tag="s")
                for hf in range(2):              # matmul moving dim max 512
                    for dt in range(8):
                        nc.tensor.matmul(
                            qkp[:, hf * 512:(hf + 1) * 512],
                            w_sb[:, dt, ft * DC:(ft + 1) * DC],
                            xts[:, dt, hf * 512:(hf + 1) * 512],
                            start=(dt == 0), stop=(dt == 7),
                        )
                _copy(nc, QK_EV, dst[:, ci * CW:(ci + 1) * CW], qkp)
            # v in feature layout then transpose k-tiles of 128 toks
            vp = psS.tile([128, CW], F32, tag="s")
            for hf in range(2):
                for dt in range(8):
                    nc.tensor.matmul(
                        vp[:, hf * 512:(hf + 1) * 512],
                        w_sb[:, dt, 2 * DC:3 * DC],
                        xts[:, dt, hf * 512:(hf + 1) * 512],
                        start=(dt == 0), stop=(dt == 7),
                    )
            vdt = F32R if AV_DT == "r" else BF16
            vT = sbvt.tile([128, CW], vdt, tag="vT")
            _copy(nc, QK_EV, vT, vp)
            for k4 in range(2):
                trp = psOP.tile([128, 4, 128], vdt, tag="op")
                for ki in range(4):
                    kk = k4 * 4 + ki
                    nc.tensor.transpose(
                        trp[:, ki, :],
                        vT[:, kk * 128:(kk + 1) * 128],
                        ident,
                    )
                kt0 = ci * 8 + k4 * 4
                _copy(nc, V_EV,
                      vv[:, kt0:kt0 + 4, :, 0:64],
                      trp.rearrange("p a (h c) -> p a h c", h=HPC))

        fetch_x(b + 1)
        # ====== phase 2 + 3: attention per (chunk, head), fused outproj ====
        attnT = sba.tile([128, T], F32R, tag="attnT")
        for ci in range(NCH):
            nsub = ci * 8                        # fully-live k-tiles
            qs = ci * CW
            all_pts = []
            for h in range(HPC):
                hs = slice(h * 64, (h + 1) * 64)
                pts = []
                for kt in range(nsub):           # sub-diagonal k-tiles
                    sp = psS.tile([128, CW], F32, tag="s")
                    for hf in range(2):
                        nc.tensor.matmul(
                            sp[:, hf * 512:(hf + 1) * 512],
                            kT[hs, kt * 128:(kt + 1) * 128],
                            qT[hs, qs + hf * 512:qs + (hf + 1) * 512],
                            start=True, stop=True,
                        )
                    pt = sbpt.tile([128, CW],
                                   F32R if AV_DT == "r" else BF16, tag="p")
                    nc.scalar.activation(pt, sp, EXP, scale=SCALE)
                    pts.append((kt, pt, 0))
                for o in range(8):               # diagonal-band k-tiles
                    kt = nsub + o
                    off = 128 * o
                    sp = psS.tile([128, CW], F32, tag="s")
                    for c0 in range(off // 512 * 512, CW, 512):
                        o0 = max(off, c0)
                        nc.tensor.matmul(
                            sp[:, o0:c0 + 512],
                            kT[hs, kt * 128:(kt + 1) * 128],
                            qT[hs, qs + o0:qs + c0 + 512],
                            start=True, stop=True,
                        )
                    pt = sbpt.tile([128, CW],
                                   F32R if AV_DT == "r" else BF16, tag="p")
                    nc.scalar.activation(pt[:, off:CW], sp[:, off:CW],
                                         EXP, scale=SCALE)
                    eng = nc.gpsimd if MASK_EV == "p" else nc.vector
                    eng.tensor_tensor(
                        out=pt[:, off:off + 128], in0=pt[:, off:off + 128],
                        in1=trimask, op=mybir.AluOpType.mult)
                    pts.append((kt, pt, off))
                all_pts.append(pts)
            # AV in 512-halves so psav tiles stay one bank; norm right after
            # each half so the next head's AV can reuse the psAV ring
            for h in range(HPC):
                pts = all_pts[h]
                for half in range(2):
                    c0 = half * 512
                    contrib = [(kt, pt, max(off, c0)) for kt, pt, off in pts
                               if max(off, c0) < c0 + 512]
                    avp = psAV.tile([128, 512], F32, tag="av")
                    for i, (kt, pt, o0) in enumerate(contrib):
                        nc.tensor.matmul(
                            avp[:, o0 - c0:512],
                            vv[:, kt, h, :],
                            pt[:, o0:c0 + 512],
                            start=(i == 0), stop=(i == len(contrib) - 1),
                        )
                    rc = sbrc.tile([128, 512], F32, tag="rc")
                    nc.vector.reciprocal(rc[0:64, :], avp[64:128, :])
                    nc.vector.tensor_tensor(
                        out=attnT[h * 64:(h + 1) * 64, qs + c0:qs + c0 + 512],
                        in0=avp[0:64, :], in1=rc[0:64, :],
                        op=mybir.AluOpType.mult,
                    )
            # ---- phase 3: out projection for this chunk's two 512-blocks ---
            for half in range(2):
                qb = ci * 2 + half
                ob = sbob.tile([128, 4, D], BF16, tag="ob")
                for tl in range(4):
                    tt = qb * 4 + tl
                    for fc in range(2):
                        op_ = psOP.tile([128, 512], F32, tag="op")
                        nc.tensor.matmul(
                            op_,
                            attnT[:, tt * 128:(tt + 1) * 128],
                            wo_sb[:, fc * 512:(fc + 1) * 512],
                            start=True, stop=True,
                        )
                        ev = OB_EV if OB_EV != "b" else ("v" if fc == 0 else "s")
                        _copy(nc, ev, ob[:, tl, fc * 512:(fc + 1) * 512], op_)
                if OUTDMA_EV == "p":
                    nc.gpsimd.dma_start(out=out4[b * 4 + qb], in_=ob)
                else:
                    nc.sync.dma_start(out=out4[b * 4 + qb], in_=ob)


def build_module():
    nc = bacc.Bacc("TRN2", target_bir_lowering=False, debug=False,
                   num_devices=NCORES)
    # x pre-tiled on host to [chunk, p, dt, tok]: one contiguous 16KB run per
    # (partition, chunk) -> 128 descriptors per chunk DMA instead of 1024
    xT = nc.declare_dram_parameter("xT", [8 * 128, 8 * CW], BF16, isOutput=False)
    wqkvT = nc.declare_dram_parameter("wqkvT", [D, 3 * DC], BF16, isOutput=False)
    woutT = nc.declare_dram_parameter("woutT", [DC, D], F32R, isOutput=False)
    trimask = nc.declare_dram_parameter(
        "trimask", [128, 128], F32 if AV_DT == "r" else BF16, isOutput=False)
    ident = nc.declare_dram_parameter(
        "ident", [128, 128], F32R if AV_DT == "r" else BF16, isOutput=False)
    vones = nc.declare_dram_parameter(
        "vones", [128, TB * HPC * 64], F32R if AV_DT == "r" else BF16,
        isOutput=False)
    # out stored as [block, p, tt, d] so each partition's write per block is
    # one contiguous 8KB run (128 descriptors per DMA); host un-permutes
    out = nc.declare_dram_parameter("out", [16 * 128, 4 * D], BF16, isOutput=True)
    with tile.TileContext(nc) as tc:
        _attention_kernel(
            tc,
            out[:].rearrange("(n p) (tt d) -> n p tt d", p=128, d=D),
            xT[:].rearrange("(g p) (dt t) -> g p dt t", p=128, t=CW),
            wqkvT[:], woutT[:], trimask[:], ident[:], vones[:],
        )
    nc.compile()
    return nc


def shard_inputs(x, w_qkv, w_out):
    """Returns per-core input maps (bf16 host-side prep)."""
    bf = ml_dtypes.bfloat16
    x_flat = np.asarray(x, np.float32).reshape(TOK, D)
    # [D, TOK] -> [chunk, p, dt, tok_local]: contiguous 16KB per (p, chunk)
    xT = np.ascontiguousarray(
        x_flat.T.reshape(8, 128, TOK // CW, CW).transpose(2, 1, 0, 3)
    ).astype(bf).reshape(8 * 128, 8 * CW)
    w_qkv = np.asarray(w_qkv, np.float32)
    w_out = np.asarray(w_out, np.float32)
    kp = np.arange(128)[:, None]
    qf = np.arange(128)[None, :]
    trimask = (kp <= qf).astype(
        np.float32 if AV_DT == "r" else bf)              # [128,128] lower-tri^T
    identm = np.eye(128, dtype=np.float32 if AV_DT == "r" else bf)
    vones = np.ones((128, TB * HPC * 64),
                    np.float32 if AV_DT == "r" else bf)
    in_maps = []
    for c in range(NCORES):
        r0 = c * DC
        wq = w_qkv[r0:r0 + DC]
        wk = w_qkv[D + r0:D + r0 + DC]
        wv = w_qkv[2 * D + r0:2 * D + r0 + DC]
        wqkvT = np.ascontiguousarray(
            np.concatenate([wq, wk, wv], axis=0).T).astype(bf)   # [D, 3*DC]
        woutT = np.ascontiguousarray(w_out[:, r0:r0 + DC].T)     # [DC, D] f32
        in_maps.append({"xT": xT, "wqkvT": wqkvT, "woutT": woutT,
                       "trimask": trimask, "ident": identm, "vones": vones})
    return in_maps


_NC_CACHE = None


def kernel(x, w_qkv, w_out):
    global _NC_CACHE, LAST_RESULTS
    if _NC_CACHE is None:
        _NC_CACHE = build_module()
    nc = _NC_CACHE
    in_maps = shard_inputs(x, w_qkv, w_out)
    os.environ["BASS_NEVER_TRACE"] = "1"
    res = run_bass_kernel_spmd(nc, in_maps, list(range(NCORES)), trace=False)
    LAST_RESULTS = res
    acc = np.zeros((16, 4, 128, D), dtype=np.float32)
    for r in res.results:
        # [block*p, tt*d] -> [block, tt, p, d]; tok = (block*4 + tt)*128 + p
        acc += r["out"].astype(np.float32).reshape(
            16, 128, 4, D).transpose(0, 2, 1, 3)
    return acc.reshape(B, T, D)



# revision 5
# speedup vs baseline: 1.1698x; 1.1698x over previous
"""Causal self-attention (B=4, T=2048, D=1024, H=16) on 8 Trainium2 cores.

Sharding: tensor-parallel over heads - 2 heads per core. Each core computes
its QKV shard, causal attention for its heads, and a partial output
projection; the host sums the 8 partials.

v2: software-pipelined emission so the PE (Tensor) engine never stalls.
The PE executes its queue in order, and any stall resets its power-state
ramp (2.4GHz only after ~3us of continuous execution, else 1.2GHz), so the
whole kernel is scheduled as ONE interleaved PE stream:

  - scores and AV matmuls run in lockstep per (q-half, k-tile) unit with a
    LAG-unit delay: the AV matmul for unit i is emitted right after the
    scores matmul of unit i+LAG, by which time the ACT exp of unit i is
    done.  Steady state: PE does scores+AV (2 passes, 0.42ns/col) while ACT
    does one exp pass (0.83ns/col) - balanced, no stalls.
  - QKV projection matmuls for chunk t+1 are ACT-free filler units injected
    every QKV_EVERY attn units of chunk t (and bulk-drained alongside the
    out-projection), hiding the ACT overhead deficit and giving the PE work
    wherever exp lags.
  - score tiles are [128, 2x512]: both heads of a q-half share one PSUM
    tile, so ONE exp instruction covers both heads (halves ACT instruction
    overhead).
  - qT/kT/vT PSUM->SBUF copies go to the otherwise-idle Pool engine; exp is
    ACT's only major load; mask/reciprocal/norm/out-proj copies go to DVE.
  - probs/V/attnT/w_out are bf16 (AV_DT=b): halves SBUF, enables DVE 2x for
    the causal masks; QKV stays bf16; scores q/k stay fp32r.

Per-core dataflow (PSUM accum fp32 everywhere):
  QKV unit stream (chunk s): per 512-half of q/k/v: 8 accumulating bf16
      matmuls -> [128,512] PSUM -> Pool copy to qT/kT [feat, tok] SBUF or
      vT; then 2x4 PE transposes -> vv [k, kt, h, 0:64] with ones in cols
      64:128 (denominator row-sum trick, memset once on Pool).
  attn stream (chunk t): per (q-half, k-tile): two 64-contract fp32r
      score matmuls (one per head) into a shared [128, 2, 512] PSUM tile;
      one exp (ACT, fused 1/8 scale) -> pt bf16; diagonal-band 128x128
      blocks masked multiplicatively in place (DVE); lagged AV bf16 matmuls
      accumulate [128,512] with ones rows giving denominators on partitions
      64:127; reciprocal * mult -> attnT (bf16).
  out-proj (chunk t): per 128-tok tile: 2 bf16 matmuls attnT^T x wout ->
      [128,512] PSUM -> DVE copy to bf16 ob -> one DMA per 512-tok block on
      the Pool DGE queue (keeps the SP queue free for x prefetches).
  Host sums the 8 partial projections.
"""

import os
import sys
from collections import deque

sys.path.insert(0, "/opt/trn_rl_repo")

import numpy as np
import ml_dtypes
from contextlib import ExitStack

import concourse.bass as bass
import concourse.mybir as mybir
import concourse.tile as tile
from concourse import bacc
from concourse.bass_utils import run_bass_kernel_spmd

B, T, D, H, HD = 4, 2048, 1024, 16, 64
NCORES = 8
HPC = H // NCORES          # heads per core = 2
DC = HPC * HD              # per-core feature width = 128
TOK = B * T                # 8192
TB = T // 128              # k-tiles per batch = 16
CW = 1024                  # q-chunk width
NCH = T // CW              # chunks per batch = 2
F32 = mybir.dt.float32
F32R = mybir.dt.float32r
BF16 = mybir.dt.bfloat16
EXP = mybir.ActivationFunctionType.Exp
SCALE = 1.0 / 8.0          # 1/sqrt(HD)

LAST_RESULTS = None


def _env(name, dflt):
    return os.environ.get(name, dflt)


QK_EV = _env("K_QK_EV", "v")       # qT/kT/vT PSUM->SBUF copies (NOT p: Pool cannot read PSUM)
OB_EV = _env("K_OB_EV", "v")       # outproj copies: v=DVE, s=ACT, b=alternate
V_EV = _env("K_V_EV", "v")         # v transpose-pack copies: v=DVE
MASK_EV = _env("K_MASK_EV", "v")   # tri-mask mult: v=DVE, p=Pool
OUTDMA_EV = _env("K_OUTDMA_EV", "p")  # out DMA issue queue: p=Pool, s=SP
AV_DT = _env("K_AV_DT", "b")       # probs/V dtype: r=fp32r, b=bf16
LAG = int(_env("K_LAG", "2"))      # attn units between scores and their AV
QKV_EVERY = int(_env("K_QKV_EVERY", "8"))  # qkv filler cadence (attn units)
PT_BUFS = int(_env("K_PT_BUFS", "6"))
XTS_BUFS = int(_env("K_XTS_BUFS", "3"))


def _copy(nc, ev, out, in_):
    if ev == "v":
        nc.vector.tensor_copy(out, in_)
    elif ev == "p":
        nc.gpsimd.tensor_copy(out, in_)
    else:
        nc.scalar.copy(out, in_)


def _attention_kernel(tc, out4, xTr, wqkvT, woutT, trimaskd, identd):
    nc = tc.nc
    with ExitStack() as ctx:
        const = ctx.enter_context(tc.tile_pool(name="const", bufs=1))
        sbqk = ctx.enter_context(tc.tile_pool(name="sbqk", bufs=2))
        sbvv = ctx.enter_context(tc.tile_pool(name="sbvv", bufs=1))
        sbvt = ctx.enter_context(tc.tile_pool(name="sbvt", bufs=2))
        sbx = ctx.enter_context(tc.tile_pool(name="sbx", bufs=XTS_BUFS))
        sbpt = ctx.enter_context(tc.tile_pool(name="sbpt", bufs=PT_BUFS))
        sba = ctx.enter_context(tc.tile_pool(name="sba", bufs=2))
        sbrc = ctx.enter_context(tc.tile_pool(name="sbrc", bufs=2))
        sbob = ctx.enter_context(tc.tile_pool(name="sbob", bufs=2))
        psS = ctx.enter_context(tc.tile_pool(name="psS", bufs=2, space="PSUM"))
        psQK = ctx.enter_context(tc.tile_pool(name="psQK", bufs=2, space="PSUM"))
        psAV = ctx.enter_context(tc.tile_pool(name="psAV", bufs=2, space="PSUM"))

        PDT = BF16 if AV_DT == "b" else F32R

        # ---- constants ----
        w_sb = const.tile([128, 8, 3 * DC], BF16, tag="wqkv")
        nc.sync.dma_start(out=w_sb, in_=wqkvT.rearrange("(dt p) f -> p dt f", p=128))
        wo_sb = const.tile([128, D], PDT, tag="wout")
        nc.sync.dma_start(out=wo_sb, in_=woutT)
        trimask = const.tile([128, 128], F32 if AV_DT == "r" else BF16,
                             tag="trimask")
        nc.sync.dma_start(out=trimask, in_=trimaskd)
        ident = const.tile([128, 128], PDT, tag="ident")
        nc.sync.dma_start(out=ident, in_=identd)

        pools = (sbqk, sbvt, sbx, sbpt, sba, sbrc, sbob, psS, psQK, psAV)

        # vv double-buffered manually (batch parity); ones cols written once
        # (Pool memset keeps startup off the DMA path)
        vvs = []
        for pb in range(2):
            vv = sbvv.tile([128, TB, HPC, 128], PDT, tag=f"vv{pb}")
            ones_ap = vv[:, :, :, 64:128]
            if AV_DT == "r":
                ones_ap = ones_ap.bitcast(F32)
            nc.gpsimd.memset(ones_ap, 1.0)
            vvs.append(vv)

        def body():
            _kernel_body(tc, out4, xTr, w_sb, wo_sb, trimask, ident, vvs,
                         pools)

        nloop = int(os.environ.get("K_LOOP", "1"))
        if nloop > 1:
            with tc.For_i(0, nloop, 1):
                body()
        else:
            body()


def _kernel_body(tc, out4, xTr, w_sb, wo_sb, trimask, ident, vvs, pools):
    (sbqk, sbvt, sbx, sbpt, sba, sbrc, sbob, psS, psQK, psAV) = pools
    nc = tc.nc
    PDT = BF16 if AV_DT == "b" else F32R

    slots = [(b, ci) for b in range(B) for ci in range(NCH)]
    NS = len(slots)
    xts_tiles = {}
    qk_state = {}

    def fetch_x(s):
        if s >= NS:
            return
        b, ci = slots[s]
        xts = sbx.tile([128, 8, CW], BF16, tag="xts")
        nc.sync.dma_start(out=xts, in_=xTr[b * NCH + ci])
        xts_tiles[s] = xts

    def qkv_units(s):
        """Generator of ACT-free PE filler units: QKV projection + V pack
        for slot s. Yields after each PSUM-tile-sized unit (~1.4us PE)."""
        b, ci = slots[s]
        if ci == 0:
            qTn = sbqk.tile([128, T], F32R, tag="qT")
            kTn = sbqk.tile([128, T], F32R, tag="kT")
            qk_state[b] = (qTn, kTn)
        qT, kT = qk_state[b]
        vv = vvs[b % 2]
        xts = xts_tiles.pop(s)
        for ft, dst in ((0, qT), (1, kT)):
            for hf in range(2):
                qkp = psQK.tile([128, 512], F32, tag="qk")
                for dt in range(8):
                    nc.tensor.matmul(
                        qkp, w_sb[:, dt, ft * DC:(ft + 1) * DC],
                        xts[:, dt, hf * 512:(hf + 1) * 512],
                        start=(dt == 0), stop=(dt == 7),
                    )
                _copy(nc, QK_EV,
                      dst[:, ci * CW + hf * 512: ci * CW + (hf + 1) * 512],
                      qkp)
                yield
        vT = sbvt.tile([128, CW], PDT, tag="vT")
        for hf in range(2):
            vp = psQK.tile([128, 512], F32, tag="qk")
            for dt in range(8):
                nc.tensor.matmul(
                    vp, w_sb[:, dt, 2 * DC:3 * DC],
                    xts[:, dt, hf * 512:(hf + 1) * 512],
                    start=(dt == 0), stop=(dt == 7),
                )
            _copy(nc, QK_EV, vT[:, hf * 512:(hf + 1) * 512], vp)
            yield
        for k4 in range(2):
            trp = psQK.tile([128, 4, 128], PDT, tag="qk")
            for ki in range(4):
                kk = k4 * 4 + ki
                nc.tensor.transpose(
                    trp[:, ki, :], vT[:, kk * 128:(kk + 1) * 128], ident)
            kt0 = ci * 8 + k4 * 4
            _copy(nc, V_EV,
                  vv[:, kt0:kt0 + 4, :, 0:64],
                  trp.rearrange("p a (h c) -> p a h c", h=HPC))
            yield

    def emit_slot(t, qg):
        """Attention + out-projection for slot t, with qkv filler units for
        slot t+1 (generator qg) injected to keep the PE busy while ACT
        works off its exp backlog."""
        b, ci = slots[t]
        qs = ci * CW
        qT, kT = qk_state[b]
        vv = vvs[b % 2]
        attnT = sba.tile([128, CW], PDT, tag="attnT")
        pt_tiles = {}
        avs = {}

        def take_q(n):
            if qg is None:
                return
            for _ in range(n):
                if next(qg, "end") == "end":
                    break

        def s_emit(half, c0, kt, lo, diag):
            sp = psS.tile([128, HPC, 512], F32, tag="s")
            for h in range(HPC):
                hs = slice(h * 64, (h + 1) * 64)
                nc.tensor.matmul(
                    sp[:, h, lo:512],
                    kT[hs, kt * 128:(kt + 1) * 128],
                    qT[hs, qs + c0 + lo: qs + c0 + 512],
                    start=True, stop=True,
                )
            pt = sbpt.tile([128, HPC, 512], PDT, tag="p")
            nc.scalar.activation(pt[:, :, lo:512], sp[:, :, lo:512], EXP,
                                 scale=SCALE)
            if diag:
                eng = nc.gpsimd if MASK_EV == "p" else nc.vector
                for h in range(HPC):
                    eng.tensor_tensor(
                        out=pt[:, h, lo:lo + 128], in0=pt[:, h, lo:lo + 128],
                        in1=trimask, op=mybir.AluOpType.mult)
            pt_tiles[(half, kt)] = pt

        def av_emit(half, c0, h, i, n, kt, lo):
            if i == 0:
                avs[h] = psAV.tile([128, 512], F32, tag="av", name="avp")
            avp = avs[h]
            pt = pt_tiles[(half, kt)]
            nc.tensor.matmul(
                avp[:, lo:512], vv[:, kt, h, :], pt[:, h, lo:512],
                start=(i == 0), stop=(i == n - 1),
            )
            if i == n - 1:
                rc = sbrc.tile([128, 512], F32, tag="rc")
                nc.vector.reciprocal(rc[0:64, :], avp[64:128, :])
                nc.vector.tensor_tensor(
                    out=attnT[h * 64:(h + 1) * 64, c0:c0 + 512],
                    in0=avp[0:64, :], in1=rc[0:64, :],
                    op=mybir.AluOpType.mult,
                )

        pend = deque()
        nunit = 0
        for half in range(2):
            c0 = half * 512
            kts = [(kt, 0, False) for kt in range(ci * 8)]
            kts += [(ci * 8 + o, max(128 * o - c0, 0),
                     c0 <= 128 * o < c0 + 512)
                    for o in range(8) if 128 * o < c0 + 512]
            n = len(kts)
            for i, (kt, lo, diag) in enumerate(kts):
                s_emit(half, c0, kt, lo, diag)
                for h in range(HPC):
                    pend.append((half, c0, h, i, n, kt, lo))
                nunit += 1
                while len(pend) > HPC * LAG:
                    av_emit(*pend.popleft())
                if nunit % QKV_EVERY == 0:
                    take_q(1)
        while pend:
            av_emit(*pend.popleft())

        # ---- out-projection for this chunk's two 512-tok blocks ----
        for qb in range(2):
            ob = sbob.tile([128, 4, D], BF16, tag="ob")
            for tl in range(4):
                tt = qb * 4 + tl
                for fc in range(2):
                    op_ = psAV.tile([128, 512], F32, tag="av")
                    nc.tensor.matmul(
                        op_,
                        attnT[:, tt * 128:(tt + 1) * 128],
                        wo_sb[:, fc * 512:(fc + 1) * 512],
                        start=True, stop=True,
                    )
                    ev = OB_EV if OB_EV != "b" else ("v" if fc == 0 else "s")
                    _copy(nc, ev, ob[:, tl, fc * 512:(fc + 1) * 512], op_)
                take_q(1)
            blk = b * 4 + ci * 2 + qb
            if OUTDMA_EV == "p":
                nc.gpsimd.dma_start(out=out4[blk], in_=ob)
            else:
                nc.sync.dma_start(out=out4[blk], in_=ob)
        take_q(99)

    # ---- prologue: x for slots 0,1; QKV for slot 0 ----
    fetch_x(0)
    fetch_x(1)
    for _ in qkv_units(0):
        pass
    for t in range(NS):
        fetch_x(t + 2)
        qg = qkv_units(t + 1) if t + 1 < NS else None
        emit_slot(t, qg)


def build_module():
    nc = bacc.Bacc("TRN2", target_bir_lowering=False, debug=False,
                   num_devices=NCORES)
    # x pre-tiled on host to [chunk, p, dt, tok]: one contiguous 16KB run per
    # (partition, chunk) -> 128 descriptors per chunk DMA instead of 1024
    xT = nc.declare_dram_parameter("xT", [8 * 128, 8 * CW], BF16, isOutput=False)
    wqkvT = nc.declare_dram_parameter("wqkvT", [D, 3 * DC], BF16, isOutput=False)
    woutT = nc.declare_dram_parameter(
        "woutT", [DC, D], F32R if AV_DT == "r" else BF16, isOutput=False)
    trimask = nc.declare_dram_parameter(
        "trimask", [128, 128], F32 if AV_DT == "r" else BF16, isOutput=False)
    ident = nc.declare_dram_parameter(
        "ident", [128, 128], F32R if AV_DT == "r" else BF16, isOutput=False)
    # out stored as [block, p, tt, d] so each partition's write per block is
    # one contiguous 8KB run (128 descriptors per DMA); host un-permutes
    out = nc.declare_dram_parameter("out", [16 * 128, 4 * D], BF16, isOutput=True)
    with tile.TileContext(nc) as tc:
        _attention_kernel(
            tc,
            out[:].rearrange("(n p) (tt d) -> n p tt d", p=128, d=D),
            xT[:].rearrange("(g p) (dt t) -> g p dt t", p=128, t=CW),
            wqkvT[:], woutT[:], trimask[:], ident[:],
        )
    nc.compile()
    return nc


def shard_inputs(x, w_qkv, w_out):
    """Returns per-core input maps (bf16 host-side prep)."""
    bf = ml_dtypes.bfloat16
    x_flat = np.asarray(x, np.float32).reshape(TOK, D)
    # [D, TOK] -> [chunk, p, dt, tok_local]: contiguous 16KB per (p, chunk)
    xT = np.ascontiguousarray(
        x_flat.T.reshape(8, 128, TOK // CW, CW).transpose(2, 1, 0, 3)
    ).astype(bf).reshape(8 * 128, 8 * CW)
    w_qkv = np.asarray(w_qkv, np.float32)
    w_out = np.asarray(w_out, np.float32)
    pdt = np.float32 if AV_DT == "r" else bf
    kp = np.arange(128)[:, None]
    qf = np.arange(128)[None, :]
    trimask = (kp <= qf).astype(pdt)                     # [128,128] lower-tri^T
    identm = np.eye(128, dtype=pdt)
    in_maps = []
    for c in range(NCORES):
        r0 = c * DC
        wq = w_qkv[r0:r0 + DC]
        wk = w_qkv[D + r0:D + r0 + DC]
        wv = w_qkv[2 * D + r0:2 * D + r0 + DC]
        wqkvT = np.ascontiguousarray(
            np.concatenate([wq, wk, wv], axis=0).T).astype(bf)   # [D, 3*DC]
        woutT = np.ascontiguousarray(w_out[:, r0:r0 + DC].T).astype(pdt)
        in_maps.append({"xT": xT, "wqkvT": wqkvT, "woutT": woutT,
                       "trimask": trimask, "ident": identm})
    return in_maps


_NC_CACHE = None


def kernel(x, w_qkv, w_out):
    global _NC_CACHE, LAST_RESULTS
    if _NC_CACHE is None:
        _NC_CACHE = build_module()
    nc = _NC_CACHE
    in_maps = shard_inputs(x, w_qkv, w_out)
    os.environ["BASS_NEVER_TRACE"] = "1"
    res = run_bass_kernel_spmd(nc, in_maps, list(range(NCORES)), trace=False)
    LAST_RESULTS = res
    acc = np.zeros((16, 4, 128, D), dtype=np.float32)
    for r in res.results:
        # [block*p, tt*d] -> [block, tt, p, d]; tok = (block*4 + tt)*128 + p
        acc += r["out"].astype(np.float32).reshape(
            16, 128, 4, D).transpose(0, 2, 1, 3)
    return acc.reshape(B, T, D)


# revision 19
# speedup vs baseline: 1.3150x; 1.1241x over previous
"""Causal self-attention (B=4, T=2048, D=1024, H=16) on 8 Trainium2 cores.

Sharding: tensor-parallel over heads - 2 heads per core. Each core computes
its QKV shard, causal attention for its heads, and a partial output
projection; the host sums the 8 partials.

v2: software-pipelined emission so the PE (Tensor) engine never stalls.
The PE executes its queue in order, and any stall resets its power-state
ramp (2.4GHz only after ~3us of continuous execution, else 1.2GHz), so the
whole kernel is scheduled as ONE interleaved PE stream:

  - scores and AV matmuls run in lockstep per (q-half, k-tile) unit with a
    LAG-unit delay: the AV matmul for unit i is emitted right after the
    scores matmul of unit i+LAG, by which time the ACT exp of unit i is
    done.  Steady state: PE does scores+AV (2 passes, 0.42ns/col) while ACT
    does one exp pass (0.83ns/col) - balanced, no stalls.
  - QKV projection matmuls for chunk t+1 are ACT-free filler units injected
    every QKV_EVERY attn units of chunk t (and bulk-drained alongside the
    out-projection), hiding the ACT overhead deficit and giving the PE work
    wherever exp lags.
  - score tiles are [128, 2x512]: both heads of a q-half share one PSUM
    tile, so ONE exp instruction covers both heads (halves ACT instruction
    overhead).
  - qT/kT/vT PSUM->SBUF copies go to the otherwise-idle Pool engine; exp is
    ACT's only major load; mask/reciprocal/norm/out-proj copies go to DVE.
  - probs/V/attnT/w_out are bf16 (AV_DT=b): halves SBUF, enables DVE 2x for
    the causal masks; QKV stays bf16; scores q/k stay fp32r.

Per-core dataflow (PSUM accum fp32 everywhere):
  QKV unit stream (chunk s): per 512-half of q/k/v: 8 accumulating bf16
      matmuls -> [128,512] PSUM -> Pool copy to qT/kT [feat, tok] SBUF or
      vT; then 2x4 PE transposes -> vv [k, kt, h, 0:64] with ones in cols
      64:128 (denominator row-sum trick, memset once on Pool).
  attn stream (chunk t): per (q-half, k-tile): two 64-contract fp32r
      score matmuls (one per head) into a shared [128, 2, 512] PSUM tile;
      one exp (ACT, fused 1/8 scale) -> pt bf16; diagonal-band 128x128
      blocks masked multiplicatively in place (DVE); lagged AV bf16 matmuls
      accumulate [128,512] with ones rows giving denominators on partitions
      64:127; reciprocal * mult -> attnT (bf16).
  out-proj (chunk t): per 128-tok tile: 2 bf16 matmuls attnT^T x wout ->
      [128,512] PSUM -> DVE copy to bf16 ob -> one DMA per 512-tok block on
      the Pool DGE queue (keeps the SP queue free for x prefetches).
  Host sums the 8 partial projections.
"""

import os
import sys
from collections import deque

sys.path.insert(0, "/opt/trn_rl_repo")

import numpy as np
import ml_dtypes
from contextlib import ExitStack

import concourse.bass as bass
import concourse.mybir as mybir
import concourse.tile as tile
from concourse import bacc
from concourse.bass_utils import run_bass_kernel_spmd

B, T, D, H, HD = 4, 2048, 1024, 16, 64
NCORES = 8
HPC = H // NCORES          # heads per core = 2
DC = HPC * HD              # per-core feature width = 128
TOK = B * T                # 8192
TB = T // 128              # k-tiles per batch = 16
CW = 1024                  # q-chunk width
NCH = T // CW              # chunks per batch = 2
F32 = mybir.dt.float32
F32R = mybir.dt.float32r
BF16 = mybir.dt.bfloat16
EXP = mybir.ActivationFunctionType.Exp
SCALE = 1.0 / 8.0          # 1/sqrt(HD)

LAST_RESULTS = None


def _env(name, dflt):
    return os.environ.get(name, dflt)


QK_EV = _env("K_QK_EV", "v")       # qT/kT/vT PSUM->SBUF copies (NOT p: Pool cannot read PSUM)
OB_EV = _env("K_OB_EV", "b")       # outproj copies: v=DVE, s=ACT, b=alternate
V_EV = _env("K_V_EV", "v")         # v transpose-pack copies: v=DVE
MASK_EV = _env("K_MASK_EV", "v")   # tri-mask mult: v=DVE, p=Pool
OUTDMA_EV = _env("K_OUTDMA_EV", "p")  # out DMA issue queue: p=Pool, s=SP
AV_DT = _env("K_AV_DT", "b")       # probs/V dtype: r=fp32r, b=bf16
QK_DT = _env("K_QK_DT", "b")       # qT/kT dtype: r=fp32r, b=bf16
LAG = int(_env("K_LAG", "5"))      # attn units between scores and their AV
QKV_EVERY = int(_env("K_QKV_EVERY", "8"))  # qkv filler cadence (attn units)
PT_BUFS = int(_env("K_PT_BUFS", "6"))
XTS_BUFS = int(_env("K_XTS_BUFS", "4"))
NORM_DIV = _env("K_NORM", "r") == "d"  # r: recip+mult (d: DVE divide fails NEFF lower_dve)
SH = _env("K_SH", "0") == "1"      # per-head score tiles ([128,512] x4 ring)
PSS = int(_env("K_PSS", "2"))      # psS ring depth (in [128,2,512] tiles)
# Ablations (perf analysis only, wrong results): dma=x fetches only,
# qkv=+QKV units, noav=+scores/exp/mask, noout=everything but out-proj
ABL = _env("K_ABL", "")


def _copy(nc, ev, out, in_):
    if ev == "v":
        nc.vector.tensor_copy(out, in_)
    elif ev == "p":
        nc.gpsimd.tensor_copy(out, in_)
    else:
        nc.scalar.copy(out, in_)


def _attention_kernel(tc, out4, xTr, wqkvT, woutT, trimaskd, identd):
    nc = tc.nc
    with ExitStack() as ctx:
        const = ctx.enter_context(tc.tile_pool(name="const", bufs=1))
        sbqk = ctx.enter_context(tc.tile_pool(name="sbqk", bufs=2))
        sbvv = ctx.enter_context(tc.tile_pool(name="sbvv", bufs=1))
        sbvt = ctx.enter_context(tc.tile_pool(name="sbvt", bufs=2))
        sbx = ctx.enter_context(tc.tile_pool(name="sbx", bufs=XTS_BUFS))
        sbpt = ctx.enter_context(tc.tile_pool(name="sbpt", bufs=PT_BUFS))
        sba = ctx.enter_context(tc.tile_pool(name="sba", bufs=2))
        sbrc = ctx.enter_context(tc.tile_pool(name="sbrc", bufs=2))
        sbob = ctx.enter_context(tc.tile_pool(name="sbob", bufs=2))
        psS = ctx.enter_context(tc.tile_pool(name="psS", bufs=2, space="PSUM"))
        psQK = ctx.enter_context(tc.tile_pool(name="psQK", bufs=2, space="PSUM"))
        psAV = ctx.enter_context(tc.tile_pool(name="psAV", bufs=2, space="PSUM"))

        PDT = BF16 if AV_DT == "b" else F32R

        # ---- constants ----
        w_sb = const.tile([128, 8, 3 * DC], BF16, tag="wqkv")
        nc.sync.dma_start(out=w_sb, in_=wqkvT.rearrange("(dt p) f -> p dt f", p=128))
        wo_sb = const.tile([128, D], PDT, tag="wout")
        nc.sync.dma_start(out=wo_sb, in_=woutT)
        trimask = const.tile([128, 128], F32 if AV_DT == "r" else BF16,
                             tag="trimask")
        nc.sync.dma_start(out=trimask, in_=trimaskd)
        ident = const.tile([128, 128], PDT, tag="ident")
        nc.sync.dma_start(out=ident, in_=identd)

        pools = (sbqk, sbvt, sbx, sbpt, sba, sbrc, sbob, psS, psQK, psAV)

        # vv double-buffered manually (batch parity); ones cols written once
        # (Pool memset keeps startup off the DMA path)
        vvs = []
        for pb in range(2):
            vv = sbvv.tile([128, TB, HPC, 128], PDT, tag=f"vv{pb}")
            ones_ap = vv[:, :, :, 64:128]
            if AV_DT == "r":
                ones_ap = ones_ap.bitcast(F32)
            nc.gpsimd.memset(ones_ap, 1.0)
            vvs.append(vv)

        state = {"xts": {}, "qk": {}}
        _kernel_body(tc, out4, xTr, w_sb, wo_sb, trimask, ident, vvs,
                     pools, state, prologue=True)

        def body():
            _kernel_body(tc, out4, xTr, w_sb, wo_sb, trimask, ident, vvs,
                         pools, state)

        nloop = int(os.environ.get("K_LOOP", "1"))
        if nloop > 1:
            with tc.For_i(0, nloop, 1):
                body()
        else:
            body()


def _kernel_body(tc, out4, xTr, w_sb, wo_sb, trimask, ident, vvs, pools,
                 state, prologue=False):
    (sbqk, sbvt, sbx, sbpt, sba, sbrc, sbob, psS, psQK, psAV) = pools
    nc = tc.nc
    PDT = BF16 if AV_DT == "b" else F32R

    slots = [(b, ci) for b in range(B) for ci in range(NCH)]
    NS = len(slots)
    xts_tiles = state["xts"]
    qk_state = state["qk"]

    def fetch_x(s):
        if s >= NS:
            return
        b, ci = slots[s]
        xts = sbx.tile([128, 8, CW], BF16, tag="xts")
        nc.sync.dma_start(out=xts, in_=xTr[b * NCH + ci])
        xts_tiles[s] = xts

    def qkv_units(s):
        """Generator of ACT-free PE filler units: QKV projection + V pack
        for slot s. Yields after each PSUM-tile-sized unit (~1.4us PE)."""
        b, ci = slots[s]
        if ci == 0:
            qkdt = BF16 if QK_DT == "b" else F32R
            qTn = sbqk.tile([128, T], qkdt, tag="qT")
            kTn = sbqk.tile([128, T], qkdt, tag="kT")
            qk_state[b] = (qTn, kTn)
        qT, kT = qk_state[b]
        vv = vvs[b % 2]
        xts = xts_tiles.pop(s)
        if ABL == "dma":
            return
        for ft, dst in ((0, qT), (1, kT)):
            for hf in range(2):
                qkp = psQK.tile([128, 512], F32, tag="qk")
                for dt in range(8):
                    nc.tensor.matmul(
                        qkp, w_sb[:, dt, ft * DC:(ft + 1) * DC],
                        xts[:, dt, hf * 512:(hf + 1) * 512],
                        start=(dt == 0), stop=(dt == 7),
                    )
                _copy(nc, QK_EV,
                      dst[:, ci * CW + hf * 512: ci * CW + (hf + 1) * 512],
                      qkp)
                yield
        vT = sbvt.tile([128, CW], PDT, tag="vT")
        for hf in range(2):
            vp = psQK.tile([128, 512], F32, tag="qk")
            for dt in range(8):
                nc.tensor.matmul(
                    vp, w_sb[:, dt, 2 * DC:3 * DC],
                    xts[:, dt, hf * 512:(hf + 1) * 512],
                    start=(dt == 0), stop=(dt == 7),
                )
            _copy(nc, QK_EV, vT[:, hf * 512:(hf + 1) * 512], vp)
            yield
        for k4 in range(2):
            trp = psQK.tile([128, 4, 128], PDT, tag="qk")
            for ki in range(4):
                kk = k4 * 4 + ki
                nc.tensor.transpose(
                    trp[:, ki, :], vT[:, kk * 128:(kk + 1) * 128], ident)
            kt0 = ci * 8 + k4 * 4
            _copy(nc, V_EV,
                  vv[:, kt0:kt0 + 4, :, 0:64],
                  trp.rearrange("p a (h c) -> p a h c", h=HPC))
            yield

    def emit_slot(t, qg):
        """Attention + out-projection for slot t, with qkv filler units for
        slot t+1 (generator qg) injected to keep the PE busy while ACT
        works off its exp backlog."""
        b, ci = slots[t]
        qs = ci * CW
        qT, kT = qk_state[b]
        vv = vvs[b % 2]
        attnT = sba.tile([128, CW], PDT, tag="attnT")
        pt_tiles = {}
        avs = {}

        def take_q(n):
            if qg is None:
                return
            for _ in range(n):
                if next(qg, "end") == "end":
                    break

        def s_emit(half, c0, kt, lo, diag):
            if SH:
                # per-head [128,512] score tiles: 2x exp instrs, but psS ring
                # of 4 single-bank tiles gives the PE more run-ahead room
                pt = sbpt.tile([128, HPC, 512], PDT, tag="p")
                eng = nc.gpsimd if MASK_EV == "p" else nc.vector
                for h in range(HPC):
                    hs = slice(h * 64, (h + 1) * 64)
                    sp = psS.tile([128, 512], F32, tag="s", bufs=2 * PSS)
                    nc.tensor.matmul(
                        sp[:, lo:512],
                        kT[hs, kt * 128:(kt + 1) * 128],
                        qT[hs, qs + c0 + lo: qs + c0 + 512],
                        start=True, stop=True,
                    )
                    nc.scalar.activation(pt[:, h, lo:512], sp[:, lo:512],
                                         EXP, scale=SCALE)
                    if diag:
                        eng.tensor_tensor(
                            out=pt[:, h, lo:lo + 128],
                            in0=pt[:, h, lo:lo + 128],
                            in1=trimask, op=mybir.AluOpType.mult)
                pt_tiles[(half, kt)] = pt
                return
            sp = psS.tile([128, HPC, 512], F32, tag="s", bufs=PSS)
            for h in range(HPC):
                hs = slice(h * 64, (h + 1) * 64)
                nc.tensor.matmul(
                    sp[:, h, lo:512],
                    kT[hs, kt * 128:(kt + 1) * 128],
                    qT[hs, qs + c0 + lo: qs + c0 + 512],
                    start=True, stop=True,
                )
            pt = sbpt.tile([128, HPC, 512], PDT, tag="p")
            nc.scalar.activation(pt[:, :, lo:512], sp[:, :, lo:512], EXP,
                                 scale=SCALE)
            if diag:
                eng = nc.gpsimd if MASK_EV == "p" else nc.vector
                for h in range(HPC):
                    eng.tensor_tensor(
                        out=pt[:, h, lo:lo + 128], in0=pt[:, h, lo:lo + 128],
                        in1=trimask, op=mybir.AluOpType.mult)
            pt_tiles[(half, kt)] = pt

        def av_emit(half, c0, h, i, n, kt, lo):
            if i == 0:
                avs[h] = psAV.tile([128, 512], F32, tag="av", name="avp")
            avp = avs[h]
            pt = pt_tiles[(half, kt)]
            nc.tensor.matmul(
                avp[:, lo:512], vv[:, kt, h, :], pt[:, h, lo:512],
                start=(i == 0), stop=(i == n - 1),
            )
            if i == n - 1:
                if NORM_DIV:
                    nc.vector.tensor_tensor(
                        out=attnT[h * 64:(h + 1) * 64, c0:c0 + 512],
                        in0=avp[0:64, :], in1=avp[64:128, :],
                        op=mybir.AluOpType.divide,
                    )
                else:
                    rc = sbrc.tile([128, 512], F32, tag="rc")
                    nc.vector.reciprocal(rc[0:64, :], avp[64:128, :])
                    nc.vector.tensor_tensor(
                        out=attnT[h * 64:(h + 1) * 64, c0:c0 + 512],
                        in0=avp[0:64, :], in1=rc[0:64, :],
                        op=mybir.AluOpType.mult,
                    )

        pend = deque()
        nunit = 0
        if ABL in ("qkv", "dma"):
            take_q(99)
            return
        for half in range(2):
            c0 = half * 512
            kts = [(kt, 0, False) for kt in range(ci * 8)]
            kts += [(ci * 8 + o, max(128 * o - c0, 0),
                     c0 <= 128 * o < c0 + 512)
                    for o in range(8) if 128 * o < c0 + 512]
            n = len(kts)
            for i, (kt, lo, diag) in enumerate(kts):
                s_emit(half, c0, kt, lo, diag)
                if ABL != "noav":
                    for h in range(HPC):
                        pend.append((half, c0, h, i, n, kt, lo))
                nunit += 1
                while len(pend) > HPC * LAG:
                    av_emit(*pend.popleft())
                if nunit % QKV_EVERY == 0:
                    take_q(1)
        while pend:
            av_emit(*pend.popleft())

        if ABL in ("noav", "noout"):
            take_q(99)
            return
        # ---- out-projection for this chunk's two 512-tok blocks ----
        for qb in range(2):
            ob = sbob.tile([128, 4, D], BF16, tag="ob")
            for tl in range(4):
                tt = qb * 4 + tl
                for fc in range(2):
                    op_ = psAV.tile([128, 512], F32, tag="av")
                    nc.tensor.matmul(
                        op_,
                        attnT[:, tt * 128:(tt + 1) * 128],
                        wo_sb[:, fc * 512:(fc + 1) * 512],
                        start=True, stop=True,
                    )
                    ev = OB_EV if OB_EV != "b" else ("v" if fc == 0 else "s")
                    _copy(nc, ev, ob[:, tl, fc * 512:(fc + 1) * 512], op_)
                take_q(1)
            blk = b * 4 + ci * 2 + qb
            if OUTDMA_EV == "p":
                nc.gpsimd.dma_start(out=out4[blk], in_=ob)
            else:
                nc.sync.dma_start(out=out4[blk], in_=ob)
        take_q(99)

    # Cyclic pipeline: slot t's filler is qkv((t+1) % NS), so the body wraps
    # seamlessly across For_i iterations (slot 7's filler = next iteration's
    # first QKV chunk; ring slot phases align because ring sizes divide the
    # per-iteration allocation counts).  The prologue is emitted once,
    # outside the For_i loop.
    if prologue:
        fetch_x(0)
        fetch_x(1)
        for _ in qkv_units(0):
            pass
        return
    for t in range(NS):
        fetch_x((t + 2) % NS)
        emit_slot(t, qkv_units((t + 1) % NS))


def build_module():
    nc = bacc.Bacc("TRN2", target_bir_lowering=False, debug=False,
                   num_devices=NCORES)
    # x pre-tiled on host to [chunk, p, dt, tok]: one contiguous 16KB run per
    # (partition, chunk) -> 128 descriptors per chunk DMA instead of 1024
    xT = nc.declare_dram_parameter("xT", [8 * 128, 8 * CW], BF16, isOutput=False)
    wqkvT = nc.declare_dram_parameter("wqkvT", [D, 3 * DC], BF16, isOutput=False)
    woutT = nc.declare_dram_parameter(
        "woutT", [DC, D], F32R if AV_DT == "r" else BF16, isOutput=False)
    trimask = nc.declare_dram_parameter(
        "trimask", [128, 128], F32 if AV_DT == "r" else BF16, isOutput=False)
    ident = nc.declare_dram_parameter(
        "ident", [128, 128], F32R if AV_DT == "r" else BF16, isOutput=False)
    # out stored as [block, p, tt, d] so each partition's write per block is
    # one contiguous 8KB run (128 descriptors per DMA); host un-permutes
    out = nc.declare_dram_parameter("out", [16 * 128, 4 * D], BF16, isOutput=True)
    with tile.TileContext(nc) as tc:
        _attention_kernel(
            tc,
            out[:].rearrange("(n p) (tt d) -> n p tt d", p=128, d=D),
            xT[:].rearrange("(g p) (dt t) -> g p dt t", p=128, t=CW),
            wqkvT[:], woutT[:], trimask[:], ident[:],
        )
    nc.compile()
    return nc


def shard_inputs(x, w_qkv, w_out):
    """Returns per-core input maps (bf16 host-side prep)."""
    bf = ml_dtypes.bfloat16
    x_flat = np.asarray(x, np.float32).reshape(TOK, D)
    # [D, TOK] -> [chunk, p, dt, tok_local]: contiguous 16KB per (p, chunk)
    xT = np.ascontiguousarray(
        x_flat.T.reshape(8, 128, TOK // CW, CW).transpose(2, 1, 0, 3)
    ).astype(bf).reshape(8 * 128, 8 * CW)
    w_qkv = np.asarray(w_qkv, np.float32)
    w_out = np.asarray(w_out, np.float32)
    pdt = np.float32 if AV_DT == "r" else bf
    kp = np.arange(128)[:, None]
    qf = np.arange(128)[None, :]
    trimask = (kp <= qf).astype(pdt)                     # [128,128] lower-tri^T
    identm = np.eye(128, dtype=pdt)
    in_maps = []
    for c in range(NCORES):
        r0 = c * DC
        wq = w_qkv[r0:r0 + DC]
        wk = w_qkv[D + r0:D + r0 + DC]
        wv = w_qkv[2 * D + r0:2 * D + r0 + DC]
        wqkvT = np.ascontiguousarray(
            np.concatenate([wq, wk, wv], axis=0).T).astype(bf)   # [D, 3*DC]
        woutT = np.ascontiguousarray(w_out[:, r0:r0 + DC].T).astype(pdt)
        in_maps.append({"xT": xT, "wqkvT": wqkvT, "woutT": woutT,
                       "trimask": trimask, "ident": identm})
    return in_maps


_NC_CACHE = None


def kernel(x, w_qkv, w_out):
    global _NC_CACHE, LAST_RESULTS
    if _NC_CACHE is None:
        _NC_CACHE = build_module()
    nc = _NC_CACHE
    in_maps = shard_inputs(x, w_qkv, w_out)
    os.environ["BASS_NEVER_TRACE"] = "1"
    res = run_bass_kernel_spmd(nc, in_maps, list(range(NCORES)), trace=False)
    LAST_RESULTS = res
    acc = np.zeros((16, 4, 128, D), dtype=np.float32)
    for r in res.results:
        # [block*p, tt*d] -> [block, tt, p, d]; tok = (block*4 + tt)*128 + p
        acc += r["out"].astype(np.float32).reshape(
            16, 128, 4, D).transpose(0, 2, 1, 3)
    return acc.reshape(B, T, D)


# revision 20
# speedup vs baseline: 1.3444x; 1.0224x over previous
"""Causal self-attention (B=4, T=2048, D=1024, H=16) on 8 Trainium2 cores.

Sharding: tensor-parallel over heads - 2 heads per core. Each core computes
its QKV shard, causal attention for its heads, and a partial output
projection; the host sums the 8 partials.

v2: software-pipelined emission so the PE (Tensor) engine never stalls.
The PE executes its queue in order, and any stall resets its power-state
ramp (2.4GHz only after ~3us of continuous execution, else 1.2GHz), so the
whole kernel is scheduled as ONE interleaved PE stream:

  - scores and AV matmuls run in lockstep per (q-half, k-tile) unit with a
    LAG-unit delay: the AV matmul for unit i is emitted right after the
    scores matmul of unit i+LAG, by which time the ACT exp of unit i is
    done.  Steady state: PE does scores+AV (2 passes, 0.42ns/col) while ACT
    does one exp pass (0.83ns/col) - balanced, no stalls.
  - QKV projection matmuls for chunk t+1 are ACT-free filler units injected
    every QKV_EVERY attn units of chunk t (and bulk-drained alongside the
    out-projection), hiding the ACT overhead deficit and giving the PE work
    wherever exp lags.
  - score tiles are [128, 2x512]: both heads of a q-half share one PSUM
    tile, so ONE exp instruction covers both heads (halves ACT instruction
    overhead).
  - exp is ACT's only major load (plus half the out-proj copies);
    masks/reciprocal/norm/qkv+out-proj copies go to DVE (Pool cannot
    access PSUM on TRN2).
  - probs/V/attnT/w_out are bf16 (AV_DT=b): halves SBUF, enables DVE 2x for
    the causal masks; QKV stays bf16; scores q/k stay fp32r.

Per-core dataflow (PSUM accum fp32 everywhere):
  QKV unit stream (chunk s): per 512-half of q/k/v: 8 accumulating bf16
      matmuls -> [128,512] PSUM -> Pool copy to qT/kT [feat, tok] SBUF or
      vT (DVE); then 2x4 PE transposes -> vv [k, kt, h, 0:64] with ones in cols
      64:128 (denominator row-sum trick, memset once on Pool).
  attn stream (chunk t): per (q-half, k-tile): two 64-contract fp32r
      score matmuls (one per head) into a shared [128, 2, 512] PSUM tile;
      one exp (ACT, fused 1/8 scale) -> pt bf16; diagonal-band 128x128
      blocks masked multiplicatively in place (DVE); lagged AV bf16 matmuls
      accumulate [128,512] with ones rows giving denominators on partitions
      64:127; reciprocal * mult -> attnT (bf16).
  out-proj (chunk t): per 128-tok tile: 2 bf16 matmuls attnT^T x wout ->
      [128,512] PSUM -> DVE copy to bf16 ob -> one DMA per 512-tok block on
      the Pool DGE queue (keeps the SP queue free for x prefetches).
  Host sums the 8 partial projections.
"""

import os
import sys
from collections import deque

sys.path.insert(0, "/opt/trn_rl_repo")

import numpy as np
import ml_dtypes
from contextlib import ExitStack

import concourse.bass as bass
import concourse.mybir as mybir
import concourse.tile as tile
from concourse import bacc
from concourse.bass_utils import run_bass_kernel_spmd

B, T, D, H, HD = 4, 2048, 1024, 16, 64
NCORES = 8
HPC = H // NCORES          # heads per core = 2
DC = HPC * HD              # per-core feature width = 128
TOK = B * T                # 8192
TB = T // 128              # k-tiles per batch = 16
CW = 1024                  # q-chunk width
NCH = T // CW              # chunks per batch = 2
F32 = mybir.dt.float32
F32R = mybir.dt.float32r
BF16 = mybir.dt.bfloat16
EXP = mybir.ActivationFunctionType.Exp
SCALE = 1.0 / 8.0          # 1/sqrt(HD)

LAST_RESULTS = None


def _env(name, dflt):
    return os.environ.get(name, dflt)


QK_EV = _env("K_QK_EV", "v")       # qT/kT/vT PSUM->SBUF copies (NOT p: Pool cannot read PSUM)
OB_EV = _env("K_OB_EV", "b")       # outproj copies: v=DVE, s=ACT, b=alternate
V_EV = _env("K_V_EV", "v")         # v transpose-pack copies: v=DVE
MASK_EV = _env("K_MASK_EV", "v")   # tri-mask mult: v=DVE, p=Pool
OUTDMA_EV = _env("K_OUTDMA_EV", "p")  # out DMA issue queue: p=Pool, s=SP
AV_DT = _env("K_AV_DT", "b")       # probs/V dtype: r=fp32r, b=bf16
QK_DT = _env("K_QK_DT", "b")       # qT/kT dtype: r=fp32r, b=bf16
LAG = int(_env("K_LAG", "6"))      # attn units between scores and their AV
QKV_EVERY = int(_env("K_QKV_EVERY", "6"))  # qkv filler cadence (attn units)
PT_BUFS = int(_env("K_PT_BUFS", "6"))
XTS_BUFS = int(_env("K_XTS_BUFS", "4"))
NORM_DIV = _env("K_NORM", "r") == "d"  # r: recip+mult (d: DVE divide fails NEFF lower_dve)
SH = _env("K_SH", "0") == "1"      # per-head score tiles ([128,512] x4 ring)
PSS = int(_env("K_PSS", "2"))      # psS ring depth (in [128,2,512] tiles)
# Ablations (perf analysis only, wrong results): dma=x fetches only,
# qkv=+QKV units, noav=+scores/exp/mask, noout=everything but out-proj
ABL = _env("K_ABL", "")


def _copy(nc, ev, out, in_):
    if ev == "v":
        nc.vector.tensor_copy(out, in_)
    elif ev == "p":
        nc.gpsimd.tensor_copy(out, in_)
    else:
        nc.scalar.copy(out, in_)


def _attention_kernel(tc, out4, xTr, wqkvT, woutT, trimaskd, identd):
    nc = tc.nc
    with ExitStack() as ctx:
        const = ctx.enter_context(tc.tile_pool(name="const", bufs=1))
        sbqk = ctx.enter_context(tc.tile_pool(name="sbqk", bufs=2))
        sbvv = ctx.enter_context(tc.tile_pool(name="sbvv", bufs=1))
        sbvt = ctx.enter_context(tc.tile_pool(name="sbvt", bufs=2))
        sbx = ctx.enter_context(tc.tile_pool(name="sbx", bufs=XTS_BUFS))
        sbpt = ctx.enter_context(tc.tile_pool(name="sbpt", bufs=PT_BUFS))
        sba = ctx.enter_context(tc.tile_pool(name="sba", bufs=2))
        sbrc = ctx.enter_context(tc.tile_pool(name="sbrc", bufs=2))
        sbob = ctx.enter_context(tc.tile_pool(name="sbob", bufs=2))
        psS = ctx.enter_context(tc.tile_pool(name="psS", bufs=2, space="PSUM"))
        psQK = ctx.enter_context(tc.tile_pool(name="psQK", bufs=2, space="PSUM"))
        psAV = ctx.enter_context(tc.tile_pool(name="psAV", bufs=2, space="PSUM"))

        PDT = BF16 if AV_DT == "b" else F32R

        # ---- constants ----
        w_sb = const.tile([128, 8, 3 * DC], BF16, tag="wqkv")
        nc.sync.dma_start(out=w_sb, in_=wqkvT.rearrange("(dt p) f -> p dt f", p=128))
        wo_sb = const.tile([128, D], PDT, tag="wout")
        nc.sync.dma_start(out=wo_sb, in_=woutT)
        trimask = const.tile([128, 128], F32 if AV_DT == "r" else BF16,
                             tag="trimask")
        nc.sync.dma_start(out=trimask, in_=trimaskd)
        ident = const.tile([128, 128], PDT, tag="ident")
        nc.sync.dma_start(out=ident, in_=identd)

        pools = (sbqk, sbvt, sbx, sbpt, sba, sbrc, sbob, psS, psQK, psAV)

        # vv double-buffered manually (batch parity); ones cols written once
        # (Pool memset keeps startup off the DMA path)
        vvs = []
        for pb in range(2):
            vv = sbvv.tile([128, TB, HPC, 128], PDT, tag=f"vv{pb}")
            ones_ap = vv[:, :, :, 64:128]
            if AV_DT == "r":
                ones_ap = ones_ap.bitcast(F32)
            nc.gpsimd.memset(ones_ap, 1.0)
            vvs.append(vv)

        state = {"xts": {}, "qk": {}}
        _kernel_body(tc, out4, xTr, w_sb, wo_sb, trimask, ident, vvs,
                     pools, state, prologue=True)

        def body():
            _kernel_body(tc, out4, xTr, w_sb, wo_sb, trimask, ident, vvs,
                         pools, state)

        nloop = int(os.environ.get("K_LOOP", "1"))
        if nloop > 1:
            with tc.For_i(0, nloop, 1):
                body()
        else:
            body()


def _kernel_body(tc, out4, xTr, w_sb, wo_sb, trimask, ident, vvs, pools,
                 state, prologue=False):
    (sbqk, sbvt, sbx, sbpt, sba, sbrc, sbob, psS, psQK, psAV) = pools
    nc = tc.nc
    PDT = BF16 if AV_DT == "b" else F32R

    slots = [(b, ci) for b in range(B) for ci in range(NCH)]
    NS = len(slots)
    xts_tiles = state["xts"]
    qk_state = state["qk"]

    def fetch_x(s):
        if s >= NS:
            return
        b, ci = slots[s]
        xts = sbx.tile([128, 8, CW], BF16, tag="xts")
        nc.sync.dma_start(out=xts, in_=xTr[b * NCH + ci])
        xts_tiles[s] = xts

    def qkv_units(s):
        """Generator of ACT-free PE filler units: QKV projection + V pack
        for slot s. Yields after each PSUM-tile-sized unit (~1.4us PE)."""
        b, ci = slots[s]
        if ci == 0:
            qkdt = BF16 if QK_DT == "b" else F32R
            qTn = sbqk.tile([128, T], qkdt, tag="qT")
            kTn = sbqk.tile([128, T], qkdt, tag="kT")
            qk_state[b] = (qTn, kTn)
        qT, kT = qk_state[b]
        vv = vvs[b % 2]
        xts = xts_tiles.pop(s)
        if ABL == "dma":
            return
        for ft, dst in ((0, qT), (1, kT)):
            for hf in range(2):
                qkp = psQK.tile([128, 512], F32, tag="qk")
                for dt in range(8):
                    nc.tensor.matmul(
                        qkp, w_sb[:, dt, ft * DC:(ft + 1) * DC],
                        xts[:, dt, hf * 512:(hf + 1) * 512],
                        start=(dt == 0), stop=(dt == 7),
                    )
                _copy(nc, QK_EV,
                      dst[:, ci * CW + hf * 512: ci * CW + (hf + 1) * 512],
                      qkp)
                yield
        vT = sbvt.tile([128, CW], PDT, tag="vT")
        for hf in range(2):
            vp = psQK.tile([128, 512], F32, tag="qk")
            for dt in range(8):
                nc.tensor.matmul(
                    vp, w_sb[:, dt, 2 * DC:3 * DC],
                    xts[:, dt, hf * 512:(hf + 1) * 512],
                    start=(dt == 0), stop=(dt == 7),
                )
            _copy(nc, QK_EV, vT[:, hf * 512:(hf + 1) * 512], vp)
            yield
        for k4 in range(2):
            trp = psQK.tile([128, 4, 128], PDT, tag="qk")
            for ki in range(4):
                kk = k4 * 4 + ki
                nc.tensor.transpose(
                    trp[:, ki, :], vT[:, kk * 128:(kk + 1) * 128], ident)
            kt0 = ci * 8 + k4 * 4
            _copy(nc, V_EV,
                  vv[:, kt0:kt0 + 4, :, 0:64],
                  trp.rearrange("p a (h c) -> p a h c", h=HPC))
            yield

    def emit_slot(t, qg):
        """Attention + out-projection for slot t, with qkv filler units for
        slot t+1 (generator qg) injected to keep the PE busy while ACT
        works off its exp backlog."""
        b, ci = slots[t]
        qs = ci * CW
        qT, kT = qk_state[b]
        vv = vvs[b % 2]
        attnT = sba.tile([128, CW], PDT, tag="attnT")
        pt_tiles = {}
        avs = {}

        def take_q(n):
            if qg is None:
                return
            for _ in range(n):
                if next(qg, "end") == "end":
                    break

        def s_emit(half, c0, kt, lo, diag):
            if SH:
                # per-head [128,512] score tiles: 2x exp instrs, but psS ring
                # of 4 single-bank tiles gives the PE more run-ahead room
                pt = sbpt.tile([128, HPC, 512], PDT, tag="p")
                eng = nc.gpsimd if MASK_EV == "p" else nc.vector
                for h in range(HPC):
                    hs = slice(h * 64, (h + 1) * 64)
                    sp = psS.tile([128, 512], F32, tag="s", bufs=2 * PSS)
                    nc.tensor.matmul(
                        sp[:, lo:512],
                        kT[hs, kt * 128:(kt + 1) * 128],
                        qT[hs, qs + c0 + lo: qs + c0 + 512],
                        start=True, stop=True,
                    )
                    nc.scalar.activation(pt[:, h, lo:512], sp[:, lo:512],
                                         EXP, scale=SCALE)
                    if diag:
                        eng.tensor_tensor(
                            out=pt[:, h, lo:lo + 128],
                            in0=pt[:, h, lo:lo + 128],
                            in1=trimask, op=mybir.AluOpType.mult)
                pt_tiles[(half, kt)] = pt
                return
            sp = psS.tile([128, HPC, 512], F32, tag="s", bufs=PSS)
            for h in range(HPC):
                hs = slice(h * 64, (h + 1) * 64)
                nc.tensor.matmul(
                    sp[:, h, lo:512],
                    kT[hs, kt * 128:(kt + 1) * 128],
                    qT[hs, qs + c0 + lo: qs + c0 + 512],
                    start=True, stop=True,
                )
            pt = sbpt.tile([128, HPC, 512], PDT, tag="p")
            nc.scalar.activation(pt[:, :, lo:512], sp[:, :, lo:512], EXP,
                                 scale=SCALE)
            if diag:
                eng = nc.gpsimd if MASK_EV == "p" else nc.vector
                for h in range(HPC):
                    eng.tensor_tensor(
                        out=pt[:, h, lo:lo + 128], in0=pt[:, h, lo:lo + 128],
                        in1=trimask, op=mybir.AluOpType.mult)
            pt_tiles[(half, kt)] = pt

        def av_emit(half, c0, h, i, n, kt, lo):
            if i == 0:
                avs[h] = psAV.tile([128, 512], F32, tag="av", name="avp")
            avp = avs[h]
            pt = pt_tiles[(half, kt)]
            nc.tensor.matmul(
                avp[:, lo:512], vv[:, kt, h, :], pt[:, h, lo:512],
                start=(i == 0), stop=(i == n - 1),
            )
            if i == n - 1:
                if NORM_DIV:
                    nc.vector.tensor_tensor(
                        out=attnT[h * 64:(h + 1) * 64, c0:c0 + 512],
                        in0=avp[0:64, :], in1=avp[64:128, :],
                        op=mybir.AluOpType.divide,
                    )
                else:
                    rc = sbrc.tile([128, 512], F32, tag="rc")
                    nc.vector.reciprocal(rc[0:64, :], avp[64:128, :])
                    nc.vector.tensor_tensor(
                        out=attnT[h * 64:(h + 1) * 64, c0:c0 + 512],
                        in0=avp[0:64, :], in1=rc[0:64, :],
                        op=mybir.AluOpType.mult,
                    )

        pend = deque()
        nunit = 0
        if ABL in ("qkv", "dma"):
            take_q(99)
            return
        for half in range(2):
            c0 = half * 512
            kts = [(kt, 0, False) for kt in range(ci * 8)]
            kts += [(ci * 8 + o, max(128 * o - c0, 0),
                     c0 <= 128 * o < c0 + 512)
                    for o in range(8) if 128 * o < c0 + 512]
            n = len(kts)
            for i, (kt, lo, diag) in enumerate(kts):
                s_emit(half, c0, kt, lo, diag)
                if ABL != "noav":
                    for h in range(HPC):
                        pend.append((half, c0, h, i, n, kt, lo))
                nunit += 1
                while len(pend) > HPC * LAG:
                    av_emit(*pend.popleft())
                if nunit % QKV_EVERY == 0:
                    take_q(1)
        while pend:
            av_emit(*pend.popleft())

        if ABL in ("noav", "noout"):
            take_q(99)
            return
        # ---- out-projection for this chunk's two 512-tok blocks ----
        for qb in range(2):
            ob = sbob.tile([128, 4, D], BF16, tag="ob")
            for tl in range(4):
                tt = qb * 4 + tl
                for fc in range(2):
                    op_ = psAV.tile([128, 512], F32, tag="av")
                    nc.tensor.matmul(
                        op_,
                        attnT[:, tt * 128:(tt + 1) * 128],
                        wo_sb[:, fc * 512:(fc + 1) * 512],
                        start=True, stop=True,
                    )
                    ev = OB_EV if OB_EV != "b" else ("v" if fc == 0 else "s")
                    _copy(nc, ev, ob[:, tl, fc * 512:(fc + 1) * 512], op_)
                take_q(1)
            blk = b * 4 + ci * 2 + qb
            if OUTDMA_EV == "p":
                nc.gpsimd.dma_start(out=out4[blk], in_=ob)
            else:
                nc.sync.dma_start(out=out4[blk], in_=ob)
        take_q(99)

    # Cyclic pipeline: slot t's filler is qkv((t+1) % NS), so the body wraps
    # seamlessly across For_i iterations (slot 7's filler = next iteration's
    # first QKV chunk; ring slot phases align because ring sizes divide the
    # per-iteration allocation counts).  The prologue is emitted once,
    # outside the For_i loop.
    if prologue:
        fetch_x(0)
        fetch_x(1)
        for _ in qkv_units(0):
            pass
        return
    for t in range(NS):
        fetch_x((t + 2) % NS)
        emit_slot(t, qkv_units((t + 1) % NS))


def build_module():
    nc = bacc.Bacc("TRN2", target_bir_lowering=False, debug=False,
                   num_devices=NCORES)
    # x pre-tiled on host to [chunk, p, dt, tok]: one contiguous 16KB run per
    # (partition, chunk) -> 128 descriptors per chunk DMA instead of 1024
    xT = nc.declare_dram_parameter("xT", [8 * 128, 8 * CW], BF16, isOutput=False)
    wqkvT = nc.declare_dram_parameter("wqkvT", [D, 3 * DC], BF16, isOutput=False)
    woutT = nc.declare_dram_parameter(
        "woutT", [DC, D], F32R if AV_DT == "r" else BF16, isOutput=False)
    trimask = nc.declare_dram_parameter(
        "trimask", [128, 128], F32 if AV_DT == "r" else BF16, isOutput=False)
    ident = nc.declare_dram_parameter(
        "ident", [128, 128], F32R if AV_DT == "r" else BF16, isOutput=False)
    # out stored as [block, p, tt, d] so each partition's write per block is
    # one contiguous 8KB run (128 descriptors per DMA); host un-permutes
    out = nc.declare_dram_parameter("out", [16 * 128, 4 * D], BF16, isOutput=True)
    with tile.TileContext(nc) as tc:
        _attention_kernel(
            tc,
            out[:].rearrange("(n p) (tt d) -> n p tt d", p=128, d=D),
            xT[:].rearrange("(g p) (dt t) -> g p dt t", p=128, t=CW),
            wqkvT[:], woutT[:], trimask[:], ident[:],
        )
    nc.compile()
    return nc


def shard_inputs(x, w_qkv, w_out):
    """Returns per-core input maps (bf16 host-side prep)."""
    bf = ml_dtypes.bfloat16
    x_flat = np.asarray(x, np.float32).reshape(TOK, D)
    # [D, TOK] -> [chunk, p, dt, tok_local]: contiguous 16KB per (p, chunk)
    xT = np.ascontiguousarray(
        x_flat.T.reshape(8, 128, TOK // CW, CW).transpose(2, 1, 0, 3)
    ).astype(bf).reshape(8 * 128, 8 * CW)
    w_qkv = np.asarray(w_qkv, np.float32)
    w_out = np.asarray(w_out, np.float32)
    pdt = np.float32 if AV_DT == "r" else bf
    kp = np.arange(128)[:, None]
    qf = np.arange(128)[None, :]
    trimask = (kp <= qf).astype(pdt)                     # [128,128] lower-tri^T
    identm = np.eye(128, dtype=pdt)
    in_maps = []
    for c in range(NCORES):
        r0 = c * DC
        wq = w_qkv[r0:r0 + DC]
        wk = w_qkv[D + r0:D + r0 + DC]
        wv = w_qkv[2 * D + r0:2 * D + r0 + DC]
        wqkvT = np.ascontiguousarray(
            np.concatenate([wq, wk, wv], axis=0).T).astype(bf)   # [D, 3*DC]
        woutT = np.ascontiguousarray(w_out[:, r0:r0 + DC].T).astype(pdt)
        in_maps.append({"xT": xT, "wqkvT": wqkvT, "woutT": woutT,
                       "trimask": trimask, "ident": identm})
    return in_maps


_NC_CACHE = None


def kernel(x, w_qkv, w_out):
    global _NC_CACHE, LAST_RESULTS
    if _NC_CACHE is None:
        _NC_CACHE = build_module()
    nc = _NC_CACHE
    in_maps = shard_inputs(x, w_qkv, w_out)
    os.environ["BASS_NEVER_TRACE"] = "1"
    res = run_bass_kernel_spmd(nc, in_maps, list(range(NCORES)), trace=False)
    LAST_RESULTS = res
    acc = np.zeros((16, 4, 128, D), dtype=np.float32)
    for r in res.results:
        # [block*p, tt*d] -> [block, tt, p, d]; tok = (block*4 + tt)*128 + p
        acc += r["out"].astype(np.float32).reshape(
            16, 128, 4, D).transpose(0, 2, 1, 3)
    return acc.reshape(B, T, D)
